# revision 25
# baseline (speedup 1.0000x reference)
"""Trainium2 Bass kernel for DynamicSpatialAttention.

reference semantics (per sample b):
  pooled = x.mean((2,3))                       [C]
  z      = relu(pooled @ w1 + b1)              [C]
  kern   = (z @ w2 + b2).reshape(3,3)          per-sample 3x3 kernel
  m      = x.mean(1)                           [H,W]   channel-mean map
  att    = sigmoid(conv2d(m, kern, pad=1))     [H,W]
  out    = x * att[None]

Distribution: data-parallel over batch B across 8 NeuronCores (4 samples
per core, fully independent -- no collectives).

Per-core dataflow (3-queue, software-pipelined): x is streamed in as
f32 [128ch, 4096px] 2MB chunks split across BOTH HWDGE rings (sync/SP
for channel half 0, scalar/ACT for half 1) -- the two hardware queues
carry reads (plus the last sample's stores).  As each chunk lands,
ScalarE copies it to a resident bf16 tile with accum_out accumulating
the spatial sums for the pooled vector; the f32 tile dies at the copy,
so reads never wait on stores.  TensorE matmuls with a shifted one-hot
column strip accumulate the channel-sum map m from the bf16 copies
into a [32, 512] PSUM tile.  The tiny kernel-generator matmuls
(z = relu(pooled@w1+b1), kern = z@w2+b2, relu/scales on VectorE to
keep the ACT FIFO clear) produce the per-sample 3x3 kernel, broadcast
to all partitions and folded into three tridiagonal banded matrices
T_dx; the 3x3 conv is three TensorE matmuls (bands = vertical taps
with implicit zero padding, PSUM column-offset accumulation =
horizontal taps), then a ScalarE sigmoid straight to bf16 (|err|
~2e-3 << 2e-2 tolerance).  s is staged onto partition 0 in one HWDGE
DMA; K=1 ones-matmuls broadcast 1024-px units into PSUM, ScalarE
copies each unit to SBUF bf16 (exact) so the in-place VectorE multiply
runs in the 2x all-16-bit DVE mode, and SWDGE (gpsimd) cast-DMAs
widen bf16 -> f32 straight to HBM -- a third dma queue, so writes
never contend with the read rings and SBUF fabric traffic drops to
96MB/core.  Emission is chunk-interleaved across samples (phase C of
sample b with phase A of b+1, next-sample read triggers hoisted ahead
of the barrier chain) so every engine FIFO alternates and the read
rings stream through the per-sample kernel-generation barrier.  The
last sample splits per chunk: half 0 stays bf16-in-place + SWDGE cast
store, half 1 multiplies f32 into the idle xin pool and stores on the
HWDGE rings -- all three store rings drain the tail in parallel.  HBM
traffic: read x once + write out once (~128MB/core, ~375us HBM-pair
roofline; measured ~390-437us/core)."""

import numpy as np

B, C, H, W = 32, 256, 128, 128
HW = H * W
KS = 3
N_CORES = 8
BS = B // N_CORES


def build_nc(bs=BS, c=C, h=H, w=W):
    import concourse.bass as bass  # noqa: F401
    import concourse.tile as tile
    from concourse import bacc, mybir
    from concourse.masks import make_identity

    f32 = mybir.dt.float32
    bf16 = mybir.dt.bfloat16
    AX = mybir.AxisListType
    AF = mybir.ActivationFunctionType

    hw = h * w
    assert c == 256, "kernel assumes 2 channel halves of 128"
    QW = 512                      # chansum matmul free width (msum free dim)
    NQ = hw // QW                 # number of 512-wide hw chunks (rows of msum)
    assert NQ <= 32
    CH = 4096                     # x chunk free width (per channel-half)
    NCH = hw // CH                # chunks per sample-half
    QPC = CH // QW                # 512-chunks per x chunk
    BPW = 1024                    # broadcast/multiply unit width (2 PSUM banks)
    UPC = CH // BPW               # units per x chunk

    nc = bacc.Bacc("TRN2", target_bir_lowering=False, debug=False)
    x_d = nc.declare_dram_parameter("x", [bs, c, hw], f32, isOutput=False)
    w1_d = nc.declare_dram_parameter("w1", [c, c], f32, isOutput=False)
    b1_d = nc.declare_dram_parameter("b1", [c], f32, isOutput=False)
    w2_d = nc.declare_dram_parameter("w2", [c, KS * KS], f32, isOutput=False)
    b2_d = nc.declare_dram_parameter("b2", [KS * KS], f32, isOutput=False)
    out_d = nc.declare_dram_parameter("out", [bs, c, hw], f32, isOutput=True)

    with tile.TileContext(nc) as tc:
        with (
            tc.tile_pool(name="xin", bufs=3) as xin,
            tc.tile_pool(name="xbfp", bufs=14) as xbfp,
            tc.tile_pool(name="srp", bufs=1) as srp,
            tc.tile_pool(name="bpbp", bufs=3) as bpbp,
            tc.tile_pool(name="small", bufs=2) as small,
            tc.tile_pool(name="singles", bufs=1) as singles,
            tc.tile_pool(name="convt", bufs=1) as convt,
            tc.tile_pool(name="pm", bufs=3, space="PSUM") as pm,
            tc.tile_pool(name="pb", bufs=2, space="PSUM") as pb,
            tc.tile_pool(name="ps", bufs=1, space="PSUM") as ps,
        ):
            # ---- constants / weights (loaded once) ----
            estrip = singles.tile([128, 2 * NQ], bf16)
            nc.vector.memset(estrip, 0.0)
            nc.vector.memset(estrip[:, NQ : NQ + 1], 1.0)
            ones_r = singles.tile([1, 128], bf16)
            nc.vector.memset(ones_r, 1.0)
            ones_rf = singles.tile([1, 128], f32)
            nc.vector.memset(ones_rf, 1.0)
            # 0/1 diagonal masks used to build the banded conv matrices:
            # ident[h,h']=d(h'=h), d_up[h,:]=e_{h+1}, d_dn[h,:]=e_{h-1}
            ident = singles.tile([h, h], bf16)
            make_identity(nc, ident)
            d_up = singles.tile([h, h], bf16)
            d_dn = singles.tile([h, h], bf16)
            nc.vector.memset(d_up, 0.0)
            nc.vector.memset(d_dn, 0.0)
            nc.gpsimd.dma_start(out=d_up[0 : h - 1, :], in_=ident[1:h, :])
            nc.gpsimd.dma_start(out=d_dn[1:h, :], in_=ident[0 : h - 1, :])
            w1_sb = singles.tile([128, 2, c], f32)  # [i_part, i_blk, j]
            nc.gpsimd.dma_start(
                out=w1_sb, in_=w1_d.rearrange("(ib i) j -> i ib j", ib=2)
            )
            # fold the 1/(h*w) of the global avg pool into w1 once (on
            # VectorE: the ACT queue must not stall behind the weight load
            # while the first read triggers queue up)
            nc.vector.tensor_scalar_mul(out=w1_sb, in0=w1_sb, scalar1=1.0 / hw)
            w2_sb = singles.tile([128, 2, KS * KS], f32)  # [j_part, j_blk, t]
            nc.gpsimd.dma_start(
                out=w2_sb, in_=w2_d.rearrange("(jb j) t -> j jb t", jb=2)
            )
            b1_sb = singles.tile([128, 2], f32)
            nc.gpsimd.dma_start(
                out=b1_sb, in_=b1_d.rearrange("(jb j) -> j jb", jb=2)
            )
            b2_sb = singles.tile([1, KS * KS], f32)
            nc.gpsimd.dma_start(
                out=b2_sb, in_=b2_d.rearrange("(o t) -> o t", o=1)
            )

            n_mm = 2 * NCH * QPC
            NU = hw // BPW
            state = {}

            def start_sample(b):
                state[b] = {
                    "xbt": {},
                    "pre": {},
                    "msum": pm.tile([NQ, QW], f32, tag="msum", name="msum"),
                    "parts": small.tile([128, 16], f32, tag="parts", name="parts"),
                    "i_mm": 0,
                }

            def emit_reads(b, q):
                # issue both ring read triggers for chunk q; they only wait
                # on xin slots, so hoisting them ahead of the barrier chain
                # keeps both read rings streaming through it
                st = state[b]
                tt = {}
                for hh in range(2):
                    t = xin.tile([128, CH], f32, tag="x", name="xt")
                    in_eng = nc.sync if hh == 0 else nc.scalar
                    in_eng.dma_start(
                        out=t,
                        in_=x_d[
                            b, 128 * hh : 128 * (hh + 1), CH * q : CH * (q + 1)
                        ],
                    )
                    tt[hh] = t
                st["pre"][q] = tt

            def emit_a_chunk(b, q):
                # bf16-copy chunk q (both halves) with pooled accumulation,
                # chansum matmuls.  The f32 tile dies at the copy, so reads
                # never wait on stores.
                st = state[b]
                if q not in st["pre"]:
                    emit_reads(b, q)
                tt = st["pre"].pop(q)
                for hh in range(2):
                    t = tt[hh]
                    xb = xbfp.tile([128, CH], bf16, tag="xbf", name="xbf")
                    nc.scalar.activation(
                        out=xb,
                        in_=t,
                        func=AF.Copy,
                        accum_out=st["parts"][:, hh * NCH + q : hh * NCH + q + 1],
                    )
                    st["xbt"][(hh, q)] = xb
                    for s in range(QPC):
                        Q = QPC * q + s
                        nc.tensor.matmul(
                            st["msum"],
                            estrip[:, NQ - Q : 2 * NQ - Q],
                            xb[:, QW * s : QW * (s + 1)],
                            start=(st["i_mm"] == 0),
                            stop=(st["i_mm"] == n_mm - 1),
                        )
                        st["i_mm"] += 1

            def barrier(b):
                # ---- pooled -> z -> kern -> kb ----
                # (1/hw is pre-folded into w1_sb, relu on VectorE: the
                # scalar/ACT queue must stay clear so it keeps issuing
                # bf16 copies + hh=1 read DMAs of the next sample)
                st = state[b]
                parts = st["parts"]
                pooled = small.tile([128, 2], f32, tag="pooled")
                nc.vector.reduce_sum(
                    out=pooled[:, 0:1], in_=parts[:, 0:NCH], axis=AX.X
                )
                nc.vector.reduce_sum(
                    out=pooled[:, 1:2], in_=parts[:, NCH : 2 * NCH], axis=AX.X
                )
                # m reshape only depends on msum -- emit it before the
                # V<->PE ping-pong z chain so the m_sq DMA overlaps it
                # (pure bf16 reshape -> HWDGE sync ring, idle at barrier)
                m32 = small.tile([NQ, QW], bf16, tag="m32")
                nc.vector.tensor_copy(out=m32, in_=st["msum"])
                m_sq = convt.tile([h, w], bf16, tag="msq")
                nc.sync.dma_start(out=m_sq, in_=m32)
                z_sb = small.tile([128, 2], f32, tag="z")
                for j in range(2):
                    zp = ps.tile([128, 1], f32, tag="zsmall", name="zp")
                    for i in range(2):
                        nc.tensor.matmul(
                            zp,
                            w1_sb[:, i, 128 * j : 128 * (j + 1)],
                            pooled[:, i : i + 1],
                            start=(i == 0),
                            stop=(i == 1),
                        )
                    # z = relu(zp + b1)
                    nc.vector.tensor_scalar(
                        out=z_sb[:, j : j + 1],
                        in0=zp,
                        scalar1=b1_sb[:, j : j + 1],
                        scalar2=0.0,
                        op0=mybir.AluOpType.add,
                        op1=mybir.AluOpType.max,
                    )
                kp = ps.tile([1, KS * KS], f32, tag="zsmall", name="kp")
                for j in range(2):
                    nc.tensor.matmul(
                        kp,
                        z_sb[:, j : j + 1],
                        w2_sb[:, j, :],
                        start=(j == 0),
                        stop=(j == 1),
                    )
                kern = small.tile([1, KS * KS], f32, tag="kern")
                nc.vector.tensor_add(out=kern, in0=kp, in1=b2_sb)
                kbp = ps.tile([128, KS * KS], f32, tag="zsmall", name="kbp")
                nc.tensor.matmul(kbp, ones_rf, kern, start=True, stop=True)
                kb = small.tile([128, KS * KS], f32, tag="kb")
                # fold the 1/C of the channel mean into the conv weights
                nc.vector.tensor_scalar_mul(out=kb, in0=kbp, scalar1=1.0 / c)

                # ---- m -> conv 3x3 -> sigmoid ----
                # conv2d(m, kern) as 3 banded matmuls: for each kernel
                # column dx, T_dx[h,h'] = k[h-h'+1, dx] is tridiagonal;
                # att[:, w-shifted] += T_dx.T @ m[:, w-shifted].  Vertical
                # padding is implicit in the band clipping, horizontal
                # padding in the PSUM column offsets.
                tb = convt.tile([h, h], bf16, tag="tb")
                t_mats = []
                for dx in range(3):
                    T = convt.tile([h, h], bf16, tag=f"T{dx}", name="T")
                    nc.vector.tensor_scalar_mul(
                        out=T, in0=ident, scalar1=kb[:h, 3 + dx : 4 + dx]
                    )
                    nc.vector.tensor_scalar_mul(
                        out=tb, in0=d_up, scalar1=kb[:h, dx : dx + 1]
                    )
                    nc.vector.tensor_add(out=T, in0=T, in1=tb)
                    nc.vector.tensor_scalar_mul(
                        out=tb, in0=d_dn, scalar1=kb[:h, 6 + dx : 7 + dx]
                    )
                    nc.vector.tensor_add(out=T, in0=T, in1=tb)
                    t_mats.append(T)
                attp = pm.tile([h, w], f32, tag="msum", name="attp")
                nc.tensor.matmul(attp, t_mats[1], m_sq, start=True, stop=False)
                nc.tensor.matmul(
                    attp[:, 0 : w - 1],
                    t_mats[2],
                    m_sq[:, 1:w],
                    start=False,
                    stop=False,
                )
                nc.tensor.matmul(
                    attp[:, 1:w],
                    t_mats[0],
                    m_sq[:, 0 : w - 1],
                    start=False,
                    stop=True,
                )
                # sigmoid straight to bf16 (|err| ~2e-3, well within
                # tolerance); no f32 copy, no lo part
                s_hi = convt.tile([h, w], bf16, tag="shi")
                nc.scalar.activation(out=s_hi, in_=attp, func=AF.Sigmoid)

                # stage all s rows onto partition 0 in one DMA (matmul rhs
                # must start at partition 0) so staging never queues behind
                # the bulk cast-stores on the SWDGE ring
                sr = srp.tile([1, hw], bf16, tag="srh", name="srh")
                nc.scalar.dma_start(out=sr, in_=s_hi)
                st["sr"] = sr

            def emit_c_chunk(b, q):
                # broadcast s via K=1 ones-matmuls; mid samples: copy PSUM
                # -> SBUF bf16 on ACT (exact: values already bf16) so the
                # multiply runs in the DVE 2x all-16-bit mode, in place,
                # then SWDGE cast-store bf16 -> f32 HBM.  Last sample: the
                # read rings are idle, so multiply f32 into the free xin
                # pool and store on BOTH HWDGE rings while SWDGE drains its
                # backlog in parallel.
                st = state[b]
                sr = st["sr"]
                last = b == bs - 1
                fo = None
                if last:
                    # hybrid tail: hh=0 keeps the 2x bf16 path + SWDGE cast
                    # store; hh=1 multiplies f32 into the idle xin pool and
                    # stores on the HWDGE rings -- all three store rings
                    # drain the last sample in parallel
                    fo = xin.tile([128, CH], f32, tag="x", name="fo")
                for u2 in range(UPC):
                    u = UPC * q + u2
                    bp = pb.tile([128, BPW], f32, tag="bp", name="bp")
                    for s2 in range(BPW // QW):
                        nc.tensor.matmul(
                            bp[:, QW * s2 : QW * (s2 + 1)],
                            ones_r,
                            sr[:, BPW * u + QW * s2 : BPW * u + QW * (s2 + 1)],
                            start=True,
                            stop=True,
                        )
                    off = BPW * u2
                    bpb = bpbp.tile([128, BPW], bf16, tag="bpb", name="bpb")
                    nc.scalar.activation(out=bpb, in_=bp, func=AF.Copy)
                    nc.vector.tensor_mul(
                        out=st["xbt"][(0, q)][:, off : off + BPW],
                        in0=st["xbt"][(0, q)][:, off : off + BPW],
                        in1=bpb,
                    )
                    if last:
                        nc.vector.tensor_mul(
                            out=fo[:, off : off + BPW],
                            in0=st["xbt"][(1, q)][:, off : off + BPW],
                            in1=bp,
                        )
                    else:
                        nc.vector.tensor_mul(
                            out=st["xbt"][(1, q)][:, off : off + BPW],
                            in0=st["xbt"][(1, q)][:, off : off + BPW],
                            in1=bpb,
                        )
                # SWDGE cast-DMA: bf16 SBUF -> f32 HBM (3rd queue)
                nc.gpsimd.dma_start(
                    out=out_d[b, 0:128, CH * q : CH * (q + 1)],
                    in_=st["xbt"][(0, q)],
                )
                del st["xbt"][(0, q)]
                if last:
                    out_eng = nc.sync if q % 2 == 0 else nc.scalar
                    out_eng.dma_start(
                        out=out_d[b, 128:256, CH * q : CH * (q + 1)],
                        in_=fo,
                    )
                else:
                    nc.gpsimd.dma_start(
                        out=out_d[b, 128:256, CH * q : CH * (q + 1)],
                        in_=st["xbt"][(1, q)],
                    )
                del st["xbt"][(1, q)]

            # ---- software-pipelined schedule: phase C of sample b is
            # emitted chunk-interleaved with phase A of sample b+1 so every
            # engine queue (ACT copies, PE matmuls, DMA triggers) alternates
            # between the two and nothing big blocks the FIFOs ----
            start_sample(0)
            for q in range(NCH):
                emit_a_chunk(0, q)
            for b in range(bs):
                if b + 1 < bs:
                    start_sample(b + 1)
                    emit_reads(b + 1, 0)
                barrier(b)
                for q in range(NCH):
                    emit_c_chunk(b, q)
                    if b + 1 < bs:
                        emit_a_chunk(b + 1, q)
                del state[b]

    nc.finalize()
    return nc


_NC_CACHE = {}


def _get_nc(key=(BS, C, H, W)):
    if key not in _NC_CACHE:
        _NC_CACHE[key] = build_nc(*key)
    return _NC_CACHE[key]


def kernel(x, w1, b1, w2, b2):
    from concourse.bass_utils import run_bass_kernel_spmd

    x = np.ascontiguousarray(x, dtype=np.float32)
    nc = _get_nc()
    in_maps = []
    for i in range(N_CORES):
        in_maps.append(
            {
                "x": x[i * BS : (i + 1) * BS].reshape(BS, C, HW),
                "w1": np.ascontiguousarray(w1, dtype=np.float32),
                "b1": np.ascontiguousarray(b1, dtype=np.float32),
                "w2": np.ascontiguousarray(w2, dtype=np.float32),
                "b2": np.ascontiguousarray(b2, dtype=np.float32),
            }
        )
    res = run_bass_kernel_spmd(nc, in_maps, list(range(N_CORES)))
    out = np.concatenate(
        [r["out"].reshape(BS, C, H, W) for r in res.results], axis=0
    )
    return out
